# revision 1
# baseline (speedup 1.0000x reference)
"""ColorHistogramLoss Trainium2 kernel.

Math: reference soft-histogram weight for pixel x and bin k is
    w = exp(-(x - c_k)^2 / (2 sigma^2)),  sigma = bin_width = 1/64, c_k = (k+0.5)/64
In bin units u = 64x:  w = exp(-(u - (k+0.5))^2 / 2).
With y = x - 0.5 (exact in fp32) and e_k = (k+0.5) - 32:
    t = 64y - e_k,   t^2/2 = 2048 y^2 - 64 e_k y + e_k^2/2
So per (pixel, bin):
    w = Exp( -(2048 y^2 - 64 e_k y) - e_k^2/2 )
The quadratic form rides the TensorEngine as a K=4 constant-stationary matmul
(rows = [yA, yA^2, yB, yB^2] for two images packed on 128 PSUM partitions =
2 x 64 bins), then a single ScalarEngine Exp pass with per-partition bias
-e_k^2/2 and fused accum_out produces per-chunk bin sums.  Host folds the
per-chunk partials in fp64, cumsums, normalizes, and takes the L1 mean.

Sharding: each of the 8 cores processes a 1/8 pixel-slice of all 24 images
(12 pred + 12 target); partial histogram sums are combined on host.
"""

import os

import numpy as np

N_CORES = 8
B, C, H, W = 4, 3, 256, 256
NIMG = 2 * B * C          # 24 images (12 pred + 12 target)
NPX = H * W               # 65536 pixels / image
PXC = NPX // N_CORES      # 8192 pixels / image / core
NPAIR = NIMG // 2         # 12 image pairs packed per matmul column-block
CHUNK = 512               # pixels per matmul (f32 moving-operand max)
ACHUNK = 1024             # pixels per ACT op (2 PSUM banks)
NCH = PXC // ACHUNK       # 8 ACT chunks per pair per core
NCOL = NPAIR * NCH        # 96 accumulator columns
BINS = 64
WIDE_F = NIMG * PXC // 128  # 1536 free-dim of the wide prep layout

_CACHE = {}


def _consts():
    k = np.arange(128) % 64
    c = k + 0.5
    e = c - 32.0
    stat = np.zeros((NPAIR, 4 * NIMG, 128), np.float32)
    for j in range(NPAIR):
        for half, cols in ((0, slice(0, 64)), (1, slice(64, 128))):
            i = 2 * j + half            # image index of this 64-bin half
            stat[j, 4 * i + 0, cols] = -64.0 * e[cols]   # y hi
            stat[j, 4 * i + 1, cols] = -64.0 * e[cols]   # y lo
            stat[j, 4 * i + 2, cols] = 2048.0            # y^2 hi
            stat[j, 4 * i + 3, cols] = 2048.0            # y^2 lo
    biasd = (-(e * e) / 2.0).astype(np.float32).reshape(128, 1)
    return stat, biasd


def _build():
    import concourse.bacc as bacc
    import concourse.tile as tile
    import concourse.mybir as mybir

    f32 = mybir.dt.float32
    bf16 = mybir.dt.bfloat16
    nc = bacc.Bacc("TRN2", target_bir_lowering=False, debug=False,
                   num_devices=N_CORES)

    xin = nc.dram_tensor("xin", [NIMG, PXC], f32, kind="ExternalInput")
    stat = nc.dram_tensor("stat", [NPAIR, 4 * NIMG, 128], bf16,
                          kind="ExternalInput")
    biasd = nc.dram_tensor("biasd", [128, 1], f32, kind="ExternalInput")
    prep = nc.dram_tensor("prep", [4 * NIMG, PXC], bf16)
    out = nc.dram_tensor("acc_out", [128, NCOL], f32, kind="ExternalOutput")

    with tile.TileContext(nc) as tc:
        with (
            tc.tile_pool(name="p_const", bufs=1) as cpool,
            tc.tile_pool(name="p_wide", bufs=1) as wpool,
            tc.tile_pool(name="p_pair", bufs=1) as ppool,
            tc.tile_pool(name="p_scr", bufs=2) as spool,
            tc.tile_pool(name="p_acc", bufs=1) as apool,
            tc.tile_pool(name="p_psum", bufs=3, space="PSUM") as qpool,
        ):
            stat_t = cpool.tile([4 * NIMG, NPAIR * 128], bf16)
            nc.sync.dma_start(
                out=stat_t[:].rearrange("k (j m) -> k j m", m=128),
                in_=stat[:].rearrange("j k m -> k j m"),
            )
            bias_t = cpool.tile([128, 1], f32)
            nc.sync.dma_start(out=bias_t[:], in_=biasd[:])

            # wide layout: partition p, col i*64+c  <=  xin[i, p*64+c]
            xw = wpool.tile([128, WIDE_F], f32)
            nc.sync.dma_start(
                out=xw[:].rearrange("p (i c) -> p i c", c=PXC // 128),
                in_=xin[:].rearrange("i (p c) -> p i c", p=128),
            )
            yw = wpool.tile([128, WIDE_F], f32)
            nc.vector.tensor_scalar_add(out=yw[:], in0=xw[:], scalar1=-0.5)
            ysq = wpool.tile([128, WIDE_F], f32)
            nc.vector.tensor_mul(out=ysq[:], in0=yw[:], in1=yw[:])
            # bf16 hi/lo splits: v = hi + lo; bf16 products are exact in the
            # fp32 PSUM accumulation, so the exponent keeps fp32-grade bits
            yh = wpool.tile([128, WIDE_F], bf16)
            nc.vector.tensor_copy(out=yh[:], in_=yw[:])
            yl = wpool.tile([128, WIDE_F], bf16)
            nc.vector.tensor_sub(out=yl[:], in0=yw[:], in1=yh[:])
            sh = wpool.tile([128, WIDE_F], bf16)
            nc.vector.tensor_copy(out=sh[:], in_=ysq[:])
            sl = wpool.tile([128, WIDE_F], bf16)
            nc.vector.tensor_sub(out=sl[:], in0=ysq[:], in1=sh[:])

            # prep rows 4i+f = feature f (yh,yl,sh,sl) of image i
            prep_v = prep[:].rearrange("(i four) (p c) -> four p i c",
                                       four=4, p=128)
            for f, srcv in enumerate((yh, yl, sh, sl)):
                nc.sync.dma_start(
                    out=prep_v[f],
                    in_=srcv[:].rearrange("p (i c) -> p i c", c=PXC // 128),
                )

            acc = apool.tile([128, NCOL], f32)
            # whole prep resident: [48 partitions, 8192] = 32KB/partition
            pt = ppool.tile([4 * NIMG, PXC], bf16)
            nc.sync.dma_start(out=pt[:], in_=prep[:])
            for j in range(NPAIR):
                for ch in range(NCH):
                    ps = qpool.tile([128, ACHUNK], f32, tag="ps")
                    for h in range(ACHUNK // CHUNK):
                        px0 = ACHUNK * ch + CHUNK * h
                        nc.tensor.matmul(
                            out=ps[:, CHUNK * h:CHUNK * (h + 1)],
                            lhsT=stat_t[:, 128 * j:128 * (j + 1)],
                            rhs=pt[:, px0:px0 + CHUNK],
                            start=True, stop=True,
                        )
                    scr = spool.tile([128, ACHUNK], f32, tag="scr")
                    col = NCH * j + ch
                    nc.scalar.activation(
                        out=scr[:], in_=ps[:],
                        func=mybir.ActivationFunctionType.Exp,
                        bias=bias_t[:, 0:1], scale=-1.0,
                        accum_out=acc[:, col:col + 1],
                    )
            nc.sync.dma_start(out=out[:], in_=acc[:])
    if not nc.is_finalized():
        nc.finalize()
    return nc


def _in_maps(pred, target):
    X = np.concatenate(
        [np.asarray(pred, np.float32).reshape(B * C, NPX),
         np.asarray(target, np.float32).reshape(B * C, NPX)], axis=0)
    statM, biasv = _consts()
    from ml_dtypes import bfloat16 as np_bf16
    statM = statM.astype(np_bf16)
    return [
        {"xin": np.ascontiguousarray(X[:, c * PXC:(c + 1) * PXC]),
         "stat": statM, "biasd": biasv}
        for c in range(N_CORES)
    ]


def _reduce(results):
    A = np.stack([r["acc_out"] for r in results]).astype(np.float64)
    M = A.reshape(N_CORES, 128, NPAIR, NCH).sum(axis=(0, 3))  # [128, 12]
    Hh = np.empty((NIMG, BINS), np.float64)
    for j in range(NPAIR):
        Hh[2 * j] = M[:64, j]
        Hh[2 * j + 1] = M[64:, j]
    cum = np.cumsum(Hh, axis=1)
    den = cum[:, -1:] + 1e-8
    cdf = cum / den
    loss = np.mean(np.abs(cdf[:B * C] - cdf[B * C:]))
    return np.array(loss, dtype=np.float32)


def kernel(pred: np.ndarray, target: np.ndarray) -> np.ndarray:
    if "nc" not in _CACHE:
        _CACHE["nc"] = _build()
    nc = _CACHE["nc"]
    in_maps = _in_maps(pred, target)

    from concourse.bass_utils import run_bass_kernel_spmd
    trace = bool(int(os.environ.get("KERNEL_TRACE", "0")))
    res = run_bass_kernel_spmd(nc, in_maps, core_ids=list(range(N_CORES)),
                               trace=trace)
    if res.exec_time_ns:
        _CACHE["exec_time_ns"] = res.exec_time_ns
    return _reduce(res.results)


def kernel_sim(pred: np.ndarray, target: np.ndarray):
    """Run through the CoreSim timing simulator; returns (loss, sim_ns)."""
    from concourse.bass_interp import MultiCoreSim
    nc = _build()
    in_maps = _in_maps(pred, target)
    sim = MultiCoreSim(nc, N_CORES)
    for c in range(N_CORES):
        for name, arr in in_maps[c].items():
            sim.cores[c].tensor(name)[:] = arr
    sim.simulate()
    results = [{"acc_out": np.array(sim.cores[c].tensor("acc_out"))}
               for c in range(N_CORES)]
    return _reduce(results), sim.global_time



# revision 43
# speedup vs baseline: 1.3503x; 1.3503x over previous
"""ColorHistogramLoss Trainium2 kernel (v3).

Math: reference soft-histogram weight for pixel x and bin k is
    w = exp(-(x - c_k)^2 / (2 sigma^2)),  sigma = bin_width = 1/64, c_k = (k+0.5)/64
In bin units u = 64x.  With y = x - 0.5 (exact in fp32) and e_k = (k+0.5) - 32:
    t = 64y - e_k,   t^2/2 = 2048 y^2 - 64 e_k y + e_k^2/2
So per (pixel, bin):
    w = Exp( -(2048 y^2 - 64 e_k y) - e_k^2/2 )
The quadratic form rides the TensorEngine as a K=32 constant-stationary matmul
(rows = bf16 hi/lo splits of [y, y^2] for the two images of a pair; 128 PSUM
partitions = 2 images x 64 bins), then one ScalarEngine Exp pass per 2048-px
chunk (per-partition bias -e_k^2/2, in-place in PSUM, fused accum_out) yields
per-chunk bin sums.  Host folds partials in fp64, cumsums, and takes the loss.

Pipeline: the 24 images are processed in 3 thirds of 8 images (one 32-row
feature block each).  Per third: load wide slice -> DVE features (bf16 hi/lo)
-> store to DRAM in pair-major layout -> reload column-chunks -> matmul+Exp.
Thirds overlap: third T+1 preps while third T streams through PE/ACT.

Sharding: each of the 8 cores processes a 1/8 pixel-slice of all 24 images
(12 pred + 12 target); partial histogram sums are combined on host.
"""

import os

import numpy as np

N_CORES = 8
B, C, H, W = 4, 3, 256, 256
NIMG = 2 * B * C          # 24 images (12 pred + 12 target)
NPX = H * W               # 65536 pixels / image
PXC = NPX // N_CORES      # 8192 pixels / image / core
NPAIR = NIMG // 2         # 12 image pairs (2 imgs x 64 bins = 128 partitions)
NTHIRD = 3                # thirds of 8 images = one 32-partition block each
TCOL = 512                # wide-layout columns per third (8 imgs x 64)
WIDE_F = NTHIRD * TCOL    # 1536 wide-layout free dim
CHUNK = 512               # pixels per matmul (one PSUM bank)
ACHUNK = 2048             # pixels per ACT op (4 PSUM banks, double buffered)
NCH = PXC // ACHUNK       # 4 ACT chunks per pair per core
NCOL = NPAIR * NCH        # 48 accumulator columns
BINS = 64

_CACHE = {}


def _consts():
    k = np.arange(128) % 64
    e = (k + 0.5) - 32.0
    half = np.arange(128) // 64       # which image of the pair
    # per-pair stationary: pair j lives at partition block 32(j//4) (same
    # base partition as its rhs rows), rows 8(j%4)+4h+f: feature f of
    # pair-image h (f=0,1: y hi/lo -> -64 e_k; f=2,3: y^2 hi/lo -> 2048),
    # nonzero only on that half's bin columns; other slots' rows stay 0.
    stat = np.zeros((96, NPAIR * 128), np.float32)
    for j in range(NPAIR):
        rb = 32 * (j // 4) + 8 * (j % 4)
        for h in (0, 1):
            cols = np.where(half == h)[0] + 128 * j
            stat[rb + 4 * h + 0, cols] = -64.0 * e[half == h]
            stat[rb + 4 * h + 1, cols] = -64.0 * e[half == h]
            stat[rb + 4 * h + 2, cols] = 2048.0
            stat[rb + 4 * h + 3, cols] = 2048.0
    # col 0: per-bin exp bias -e^2/2; col 1: the -0.5 recentering constant
    biasd = np.stack([-(e * e) / 2.0, np.full(128, -0.5)],
                     axis=1).astype(np.float32)
    return stat, biasd


def _build():
    import concourse.bacc as bacc
    import concourse.tile as tile
    import concourse.mybir as mybir

    f32 = mybir.dt.float32
    bf16 = mybir.dt.bfloat16
    nc = bacc.Bacc("TRN2", target_bir_lowering=False, debug=False,
                   num_devices=N_CORES)

    # wide layout: partition p, col 64i+c  <->  image i, pixel 64p+c
    xin = nc.dram_tensor("xin", [128, WIDE_F], f32, kind="ExternalInput")
    stat = nc.dram_tensor("stat", [96, NPAIR * 128], bf16,
                          kind="ExternalInput")
    biasd = nc.dram_tensor("biasd", [128, 2], f32, kind="ExternalInput")
    prep = nc.dram_tensor("prep", [96, PXC], bf16)
    out = nc.dram_tensor("acc_out", [128, NCOL + 2], f32,
                         kind="ExternalOutput")

    with tile.TileContext(nc) as tc:
        with (
            tc.tile_pool(name="p_const", bufs=1) as cpool,
            tc.tile_pool(name="p_wide", bufs=2) as wpool,
            tc.tile_pool(name="p_feat", bufs=3) as fpool,
            tc.tile_pool(name="p_pair", bufs=1) as ppool,
            tc.tile_pool(name="p_acc", bufs=1) as apool,
            tc.tile_pool(name="p_psum", bufs=2, space="PSUM") as qpool,
        ):
            # first wide input slice goes first so the DVE chain starts ASAP
            xws = []
            xw = wpool.tile([128, TCOL], f32, tag="xw0")
            nc.sync.dma_start(out=xw[:], in_=xin[:, 0:TCOL])
            xws.append(xw)

            stat_t = cpool.tile([96, NPAIR * 128], bf16)
            nc.sync.dma_start(out=stat_t[:], in_=stat[:])
            bias_t = cpool.tile([128, 2], f32)
            nc.sync.dma_start(out=bias_t[:], in_=biasd[:])
            # pre-warm the exp table so ACT_TABLE_LOAD is off the critical path
            warm = cpool.tile([128, 1], f32)
            nc.scalar.activation(out=warm[:], in_=bias_t[:, 0:1],
                                 func=mybir.ActivationFunctionType.Exp)

            acc = apool.tile([128, NCOL + 2], f32)
            # pt row 4i+f = feature f (yh,yl,sh,sl) of image i
            pt = ppool.tile([96, PXC], bf16)
            # prep DRAM view for the pair-major store: [f][p, i, c]
            prep_v = prep[:].rearrange("(i four) (p c) -> four p i c",
                                       four=4, p=128)

            # prefetch the remaining wide input slices
            for T in range(1, NTHIRD):
                xw = wpool.tile([128, TCOL], f32, tag=f"xw{T % 2}")
                nc.sync.dma_start(out=xw[:],
                                  in_=xin[:, TCOL * T:TCOL * (T + 1)])
                xws.append(xw)

            for T in range(NTHIRD):
                xw = xws[T]
                y = wpool.tile([128, TCOL], f32, tag=f"y{T % 2}")
                nc.vector.tensor_scalar_add(out=y[:], in0=xw[:], scalar1=-0.5)
                y2 = wpool.tile([128, TCOL], f32, tag=f"y2{T % 2}")
                nc.vector.tensor_mul(out=y2[:], in0=y[:], in1=y[:])
                # feature block: f at cols [512f, 512f+512); hi/lo bf16 splits
                # (v = hi + lo; products are exact in fp32 PSUM accumulation)
                fg = fpool.tile([128, 4 * TCOL], bf16, tag="fg")
                nc.vector.tensor_copy(out=fg[:, 0:TCOL], in_=y[:])
                nc.vector.tensor_sub(out=fg[:, TCOL:2 * TCOL], in0=y[:],
                                     in1=fg[:, 0:TCOL])
                nc.vector.tensor_copy(out=fg[:, 2 * TCOL:3 * TCOL], in_=y2[:])
                nc.vector.tensor_sub(out=fg[:, 3 * TCOL:4 * TCOL], in0=y2[:],
                                     in1=fg[:, 2 * TCOL:3 * TCOL])
                if T > 0:
                    # dependency gate: fg[0, f*TCOL] = (pt_prev * 0) + itself
                    # is an exact identity on one element of each feature
                    # block, but makes this third's stores wait for the
                    # previous third's first pt load — keeps them off the
                    # DMA device while the critical-path load is pending
                    p0 = 32 * (T - 1)   # same start partition as the pt read
                    gv = fg[p0:p0 + 1, 0:3 * TCOL + 1:TCOL]
                    nc.vector.scalar_tensor_tensor(
                        out=gv, in0=pt[p0:p0 + 1, 0:4],
                        scalar=0.0, in1=gv,
                        op0=mybir.AluOpType.mult, op1=mybir.AluOpType.add)
                # store features to DRAM in pair-major row layout (row 4i+f)
                for f in range(4):
                    nc.sync.dma_start(
                        out=prep_v[f][:, 8 * T:8 * (T + 1), :],
                        in_=fg[:, TCOL * f:TCOL * (f + 1)].rearrange(
                            "p (i c) -> p i c", c=64),
                    )
                # reload as [32-row block, column chunks] for the matmuls;
                # the very first chunk is further split 512+1536 so the
                # first matmul/ACT can launch as early as possible
                r0 = 32 * T
                if T == 0:
                    for cs in (slice(0, CHUNK), slice(CHUNK, 2 * CHUNK),
                               slice(2 * CHUNK, ACHUNK)):
                        nc.sync.dma_start(out=pt[r0:r0 + 32, cs],
                                          in_=prep[r0:r0 + 32, cs])
                else:
                    nc.sync.dma_start(out=pt[r0:r0 + 32, 0:ACHUNK],
                                      in_=prep[r0:r0 + 32, 0:ACHUNK])
                for ch in range(1, NCH):
                    cs = slice(ACHUNK * ch, ACHUNK * (ch + 1))
                    nc.sync.dma_start(out=pt[r0:r0 + 32, cs],
                                      in_=prep[r0:r0 + 32, cs])
            for T in range(NTHIRD):
                r0 = 32 * T
                for ch in range(NCH):
                    for js in range(4):
                        j = 4 * T + js
                        ps = qpool.tile([128, ACHUNK], f32, tag="ps")
                        head = T == 0 and ch == 0 and js == 0
                        for h in range(ACHUNK // CHUNK):
                            px0 = ACHUNK * ch + CHUNK * h
                            nc.tensor.matmul(
                                out=ps[:, CHUNK * h:CHUNK * (h + 1)],
                                lhsT=stat_t[r0:r0 + 32,
                                            128 * j:128 * (j + 1)],
                                rhs=pt[r0:r0 + 32, px0:px0 + CHUNK],
                                start=True, stop=True,
                            )
                            if head and h < 2:
                                # head ops: Exp the first 512-px pieces right
                                # away (extra accum cols, folded on host) so
                                # the ACT stream starts as early as possible
                                nc.scalar.activation(
                                    out=ps[:, CHUNK * h:CHUNK * (h + 1)],
                                    in_=ps[:, CHUNK * h:CHUNK * (h + 1)],
                                    func=mybir.ActivationFunctionType.Exp,
                                    bias=bias_t[:, 0:1], scale=-1.0,
                                    accum_out=acc[:, NCOL + h:NCOL + h + 1],
                                )
                        col = NCH * j + ch
                        lo = 2 * CHUNK if head else 0
                        nc.scalar.activation(
                            out=ps[:, lo:ACHUNK], in_=ps[:, lo:ACHUNK],
                            func=mybir.ActivationFunctionType.Exp,
                            bias=bias_t[:, 0:1], scale=-1.0,
                            accum_out=acc[:, col:col + 1],
                        )
                # drain the first two blocks' accumulator columns early so
                # only a tiny output DMA remains after the final Exp
                if T == 1:
                    nc.sync.dma_start(out=out[:, 0:32], in_=acc[:, 0:32])
            nc.sync.dma_start(out=out[:, 32:NCOL + 2],
                              in_=acc[:, 32:NCOL + 2])
    if not nc.is_finalized():
        nc.finalize()
    return nc


def _in_maps(pred, target):
    X = np.concatenate(
        [np.asarray(pred, np.float32).reshape(B * C, NPX),
         np.asarray(target, np.float32).reshape(B * C, NPX)], axis=0)
    statM, biasv = _consts()
    from ml_dtypes import bfloat16 as np_bf16
    statM = statM.astype(np_bf16)
    maps = []
    for cs in range(N_CORES):
        Xs = X[:, cs * PXC:(cs + 1) * PXC]              # [24, 8192]
        xin = np.ascontiguousarray(
            Xs.reshape(NIMG, 128, 64)                   # [i, p, c]
              .transpose(1, 0, 2)                       # [p, i, c]
              .reshape(128, WIDE_F))
        maps.append({"xin": xin, "stat": statM, "biasd": biasv})
    return maps


def _reduce(results):
    A = np.stack([r["acc_out"] for r in results]).astype(np.float64)
    A = A.sum(axis=0)
    A[:, 0] += A[:, NCOL] + A[:, NCOL + 1]   # fold head partials (j=0,ch=0)
    M = A[:, :NCOL].reshape(128, NPAIR, NCH).sum(axis=2)     # [128, 12]
    Hh = np.empty((NIMG, BINS), np.float64)
    for j in range(NPAIR):
        Hh[2 * j] = M[:64, j]
        Hh[2 * j + 1] = M[64:, j]
    cum = np.cumsum(Hh, axis=1)
    den = cum[:, -1:] + 1e-8
    cdf = cum / den
    loss = np.mean(np.abs(cdf[:B * C] - cdf[B * C:]))
    return np.array(loss, dtype=np.float32)


def kernel(pred: np.ndarray, target: np.ndarray) -> np.ndarray:
    if "nc" not in _CACHE:
        _CACHE["nc"] = _build()
    nc = _CACHE["nc"]
    in_maps = _in_maps(pred, target)

    from concourse.bass_utils import run_bass_kernel_spmd
    trace = bool(int(os.environ.get("KERNEL_TRACE", "0")))
    res = run_bass_kernel_spmd(nc, in_maps, core_ids=list(range(N_CORES)),
                               trace=trace)
    if res.exec_time_ns:
        _CACHE["exec_time_ns"] = res.exec_time_ns
    return _reduce(res.results)


def kernel_sim(pred: np.ndarray, target: np.ndarray):
    """Run through the CoreSim timing simulator; returns (loss, sim_ns)."""
    from concourse.bass_interp import MultiCoreSim
    nc = _build()
    in_maps = _in_maps(pred, target)
    sim = MultiCoreSim(nc, N_CORES)
    for c in range(N_CORES):
        for name, arr in in_maps[c].items():
            sim.cores[c].tensor(name)[:] = arr
    sim.simulate()
    results = [{"acc_out": np.array(sim.cores[c].tensor("acc_out"))}
               for c in range(N_CORES)]
    return _reduce(results), sim.global_time


# revision 47
# speedup vs baseline: 1.3965x; 1.0342x over previous
"""ColorHistogramLoss Trainium2 kernel (v3).

Math: reference soft-histogram weight for pixel x and bin k is
    w = exp(-(x - c_k)^2 / (2 sigma^2)),  sigma = bin_width = 1/64, c_k = (k+0.5)/64
In bin units u = 64x.  With y = x - 0.5 (exact in fp32) and e_k = (k+0.5) - 32:
    t = 64y - e_k,   t^2/2 = 2048 y^2 - 64 e_k y + e_k^2/2
So per (pixel, bin):
    w = Exp( -(2048 y^2 - 64 e_k y) - e_k^2/2 )
The quadratic form rides the TensorEngine as a K=32 constant-stationary matmul
(rows = bf16 hi/lo splits of [y, y^2] for the two images of a pair; 128 PSUM
partitions = 2 images x 64 bins), then one ScalarEngine Exp pass per 2048-px
chunk (per-partition bias -e_k^2/2, in-place in PSUM, fused accum_out) yields
per-chunk bin sums.  Host folds partials in fp64, cumsums, and takes the loss.

Pipeline: the 24 images are processed in 3 thirds of 8 images (one 32-row
feature block each).  Per third: load wide slice -> DVE features (bf16 hi/lo)
-> store to DRAM in pair-major layout -> reload column-chunks -> matmul+Exp.
Thirds overlap: third T+1 preps while third T streams through PE/ACT.

Sharding: each of the 8 cores processes a 1/8 pixel-slice of all 24 images
(12 pred + 12 target); partial histogram sums are combined on host.
"""

import os

import numpy as np

N_CORES = 8
B, C, H, W = 4, 3, 256, 256
NIMG = 2 * B * C          # 24 images (12 pred + 12 target)
NPX = H * W               # 65536 pixels / image
PXC = NPX // N_CORES      # 8192 pixels / image / core
NPAIR = NIMG // 2         # 12 image pairs (2 imgs x 64 bins = 128 partitions)
NTHIRD = 3                # thirds of 8 images = one 32-partition block each
TCOL = 512                # wide-layout columns per third (8 imgs x 64)
WIDE_F = NTHIRD * TCOL    # 1536 wide-layout free dim
CHUNK = 512               # pixels per matmul (one PSUM bank)
ACHUNK = 2048             # pixels per ACT op (4 PSUM banks, double buffered)
NCH = PXC // ACHUNK       # 4 ACT chunks per pair per core
NCOL = NPAIR * NCH        # 48 accumulator columns
BINS = 64

_CACHE = {}


def _consts():
    k = np.arange(128) % 64
    e = (k + 0.5) - 32.0
    half = np.arange(128) // 64       # which image of the pair
    # per-pair stationary: pair j lives at partition block 32(j//4) (same
    # base partition as its rhs rows), rows 8(j%4)+4h+f: feature f of
    # pair-image h (f=0,1: y hi/lo -> -64 e_k; f=2,3: y^2 hi/lo -> 2048),
    # nonzero only on that half's bin columns; other slots' rows stay 0.
    stat = np.zeros((96, NPAIR * 128), np.float32)
    for j in range(NPAIR):
        rb = 32 * (j // 4) + 8 * (j % 4)
        for h in (0, 1):
            cols = np.where(half == h)[0] + 128 * j
            stat[rb + 4 * h + 0, cols] = -64.0 * e[half == h]
            stat[rb + 4 * h + 1, cols] = -64.0 * e[half == h]
            stat[rb + 4 * h + 2, cols] = 2048.0
            stat[rb + 4 * h + 3, cols] = 2048.0
    # col 0: per-bin exp bias -e^2/2; col 1: the -0.5 recentering constant
    biasd = np.stack([-(e * e) / 2.0, np.full(128, -0.5)],
                     axis=1).astype(np.float32)
    return stat, biasd


def _build():
    import concourse.bacc as bacc
    import concourse.tile as tile
    import concourse.mybir as mybir

    f32 = mybir.dt.float32
    bf16 = mybir.dt.bfloat16
    nc = bacc.Bacc("TRN2", target_bir_lowering=False, debug=False,
                   num_devices=N_CORES)

    # wide layout: partition p, col 64i+c  <->  image i, pixel 64p+c
    xin = nc.dram_tensor("xin", [128, WIDE_F], f32, kind="ExternalInput")
    stat = nc.dram_tensor("stat", [96, NPAIR * 128], bf16,
                          kind="ExternalInput")
    biasd = nc.dram_tensor("biasd", [128, 2], f32, kind="ExternalInput")
    prep = nc.dram_tensor("prep", [96, PXC], bf16)
    out = nc.dram_tensor("acc_out", [128, NCOL + 2], f32,
                         kind="ExternalOutput")

    with tile.TileContext(nc) as tc:
        with (
            tc.tile_pool(name="p_const", bufs=1) as cpool,
            tc.tile_pool(name="p_wide", bufs=2) as wpool,
            tc.tile_pool(name="p_feat", bufs=3) as fpool,
            tc.tile_pool(name="p_pair", bufs=1) as ppool,
            tc.tile_pool(name="p_acc", bufs=1) as apool,
            tc.tile_pool(name="p_psum", bufs=2, space="PSUM") as qpool,
            tc.tile_pool(name="p_exq", bufs=4) as epool,
        ):
            # first wide input slice goes first so the DVE chain starts ASAP
            xws = []
            xw = wpool.tile([128, TCOL], f32, tag="xw0")
            nc.sync.dma_start(out=xw[:], in_=xin[:, 0:TCOL])
            xws.append(xw)

            stat_t = cpool.tile([96, NPAIR * 128], bf16)
            nc.sync.dma_start(out=stat_t[:], in_=stat[:])
            bias_t = cpool.tile([128, 2], f32)
            nc.sync.dma_start(out=bias_t[:], in_=biasd[:])
            # pre-warm the exp table so ACT_TABLE_LOAD is off the critical path
            warm = cpool.tile([128, 1], f32)
            nc.scalar.activation(out=warm[:], in_=bias_t[:, 0:1],
                                 func=mybir.ActivationFunctionType.Exp)

            acc = apool.tile([128, NCOL + 2], f32)
            # pt row 4i+f = feature f (yh,yl,sh,sl) of image i
            pt = ppool.tile([96, PXC], bf16)
            # prep DRAM view for the pair-major store: [f][p, i, c]
            prep_v = prep[:].rearrange("(i four) (p c) -> four p i c",
                                       four=4, p=128)

            # prefetch the remaining wide input slices
            for T in range(1, NTHIRD):
                xw = wpool.tile([128, TCOL], f32, tag=f"xw{T % 2}")
                nc.sync.dma_start(out=xw[:],
                                  in_=xin[:, TCOL * T:TCOL * (T + 1)])
                xws.append(xw)

            for T in range(NTHIRD):
                xw = xws[T]
                y = wpool.tile([128, TCOL], f32, tag=f"y{T % 2}")
                nc.vector.tensor_scalar_add(out=y[:], in0=xw[:], scalar1=-0.5)
                y2 = wpool.tile([128, TCOL], f32, tag=f"y2{T % 2}")
                nc.vector.tensor_mul(out=y2[:], in0=y[:], in1=y[:])
                # feature block: f at cols [512f, 512f+512); hi/lo bf16 splits
                # (v = hi + lo; products are exact in fp32 PSUM accumulation)
                fg = fpool.tile([128, 4 * TCOL], bf16, tag="fg")
                nc.vector.tensor_copy(out=fg[:, 0:TCOL], in_=y[:])
                nc.vector.tensor_sub(out=fg[:, TCOL:2 * TCOL], in0=y[:],
                                     in1=fg[:, 0:TCOL])
                nc.vector.tensor_copy(out=fg[:, 2 * TCOL:3 * TCOL], in_=y2[:])
                nc.vector.tensor_sub(out=fg[:, 3 * TCOL:4 * TCOL], in0=y2[:],
                                     in1=fg[:, 2 * TCOL:3 * TCOL])
                if T > 0:
                    # dependency gate: fg[0, f*TCOL] = (pt_prev * 0) + itself
                    # is an exact identity on one element of each feature
                    # block, but makes this third's stores wait for the
                    # previous third's first pt load — keeps them off the
                    # DMA device while the critical-path load is pending
                    p0 = 32 * (T - 1)   # same start partition as the pt read
                    gv = fg[p0:p0 + 1, 0:3 * TCOL + 1:TCOL]
                    nc.vector.scalar_tensor_tensor(
                        out=gv, in0=pt[p0:p0 + 1, 0:4],
                        scalar=0.0, in1=gv,
                        op0=mybir.AluOpType.mult, op1=mybir.AluOpType.add)
                # store features to DRAM in pair-major row layout (row 4i+f)
                for f in range(4):
                    nc.sync.dma_start(
                        out=prep_v[f][:, 8 * T:8 * (T + 1), :],
                        in_=fg[:, TCOL * f:TCOL * (f + 1)].rearrange(
                            "p (i c) -> p i c", c=64),
                    )
                # reload as [32-row block, column chunks] for the matmuls;
                # the very first chunk is further split 512+1536 so the
                # first matmul/ACT can launch as early as possible
                r0 = 32 * T
                if T == 0:
                    for cs in (slice(0, CHUNK), slice(CHUNK, 2 * CHUNK),
                               slice(2 * CHUNK, ACHUNK)):
                        nc.sync.dma_start(out=pt[r0:r0 + 32, cs],
                                          in_=prep[r0:r0 + 32, cs])
                else:
                    nc.sync.dma_start(out=pt[r0:r0 + 32, 0:ACHUNK],
                                      in_=prep[r0:r0 + 32, 0:ACHUNK])
                for ch in range(1, NCH):
                    cs = slice(ACHUNK * ch, ACHUNK * (ch + 1))
                    nc.sync.dma_start(out=pt[r0:r0 + 32, cs],
                                      in_=prep[r0:r0 + 32, cs])
            for T in range(NTHIRD):
                r0 = 32 * T
                for ch in range(NCH):
                    for js in range(4):
                        j = 4 * T + js
                        ps = qpool.tile([128, ACHUNK], f32, tag="ps")
                        head = T == 0 and ch == 0 and js == 0
                        for h in range(ACHUNK // CHUNK):
                            px0 = ACHUNK * ch + CHUNK * h
                            nc.tensor.matmul(
                                out=ps[:, CHUNK * h:CHUNK * (h + 1)],
                                lhsT=stat_t[r0:r0 + 32,
                                            128 * j:128 * (j + 1)],
                                rhs=pt[r0:r0 + 32, px0:px0 + CHUNK],
                                start=True, stop=True,
                            )
                            if head and h < 2:
                                # head ops: Exp the first 512-px pieces right
                                # away (extra accum cols, folded on host) so
                                # the ACT stream starts as early as possible
                                nc.scalar.activation(
                                    out=ps[:, CHUNK * h:CHUNK * (h + 1)],
                                    in_=ps[:, CHUNK * h:CHUNK * (h + 1)],
                                    func=mybir.ActivationFunctionType.Exp,
                                    bias=bias_t[:, 0:1], scale=-1.0,
                                    accum_out=acc[:, NCOL + h:NCOL + h + 1],
                                )
                        col = NCH * j + ch
                        lo = 2 * CHUNK if head else 0
                        if js == 3:
                            # 1-in-4: fused accumulate on the ScalarEngine
                            # (last in each group, so the stream's final op
                            # needs no trailing DVE reduce)
                            nc.scalar.activation(
                                out=ps[:, lo:ACHUNK], in_=ps[:, lo:ACHUNK],
                                func=mybir.ActivationFunctionType.Exp,
                                bias=bias_t[:, 0:1], scale=-1.0,
                                accum_out=acc[:, col:col + 1],
                            )
                        else:
                            # 3-in-4: Exp to SBUF bf16, pixel-sum on the DVE;
                            # skipping the ACT accumulator read shortens the
                            # ScalarEngine op from 2037 to 1892 ns
                            exq = epool.tile([128, ACHUNK], bf16, tag="exq")
                            nc.scalar.activation(
                                out=exq[:, 0:ACHUNK - lo], in_=ps[:, lo:ACHUNK],
                                func=mybir.ActivationFunctionType.Exp,
                                bias=bias_t[:, 0:1], scale=-1.0,
                            )
                            nc.vector.reduce_sum(
                                out=acc[:, col:col + 1],
                                in_=exq[:, 0:ACHUNK - lo],
                                axis=mybir.AxisListType.X,
                            )
                # drain the first two blocks' accumulator columns early so
                # only a tiny output DMA remains after the final Exp
                if T == 1:
                    nc.sync.dma_start(out=out[:, 0:32], in_=acc[:, 0:32])
            nc.sync.dma_start(out=out[:, 32:NCOL + 2],
                              in_=acc[:, 32:NCOL + 2])
    if not nc.is_finalized():
        nc.finalize()
    return nc


def _in_maps(pred, target):
    X = np.concatenate(
        [np.asarray(pred, np.float32).reshape(B * C, NPX),
         np.asarray(target, np.float32).reshape(B * C, NPX)], axis=0)
    statM, biasv = _consts()
    from ml_dtypes import bfloat16 as np_bf16
    statM = statM.astype(np_bf16)
    maps = []
    for cs in range(N_CORES):
        Xs = X[:, cs * PXC:(cs + 1) * PXC]              # [24, 8192]
        xin = np.ascontiguousarray(
            Xs.reshape(NIMG, 128, 64)                   # [i, p, c]
              .transpose(1, 0, 2)                       # [p, i, c]
              .reshape(128, WIDE_F))
        maps.append({"xin": xin, "stat": statM, "biasd": biasv})
    return maps


def _reduce(results):
    A = np.stack([r["acc_out"] for r in results]).astype(np.float64)
    A = A.sum(axis=0)
    A[:, 0] += A[:, NCOL] + A[:, NCOL + 1]   # fold head partials (j=0,ch=0)
    M = A[:, :NCOL].reshape(128, NPAIR, NCH).sum(axis=2)     # [128, 12]
    Hh = np.empty((NIMG, BINS), np.float64)
    for j in range(NPAIR):
        Hh[2 * j] = M[:64, j]
        Hh[2 * j + 1] = M[64:, j]
    cum = np.cumsum(Hh, axis=1)
    den = cum[:, -1:] + 1e-8
    cdf = cum / den
    loss = np.mean(np.abs(cdf[:B * C] - cdf[B * C:]))
    return np.array(loss, dtype=np.float32)


def kernel(pred: np.ndarray, target: np.ndarray) -> np.ndarray:
    if "nc" not in _CACHE:
        _CACHE["nc"] = _build()
    nc = _CACHE["nc"]
    in_maps = _in_maps(pred, target)

    from concourse.bass_utils import run_bass_kernel_spmd
    trace = bool(int(os.environ.get("KERNEL_TRACE", "0")))
    res = run_bass_kernel_spmd(nc, in_maps, core_ids=list(range(N_CORES)),
                               trace=trace)
    if res.exec_time_ns:
        _CACHE["exec_time_ns"] = res.exec_time_ns
    return _reduce(res.results)


def kernel_sim(pred: np.ndarray, target: np.ndarray):
    """Run through the CoreSim timing simulator; returns (loss, sim_ns)."""
    from concourse.bass_interp import MultiCoreSim
    nc = _build()
    in_maps = _in_maps(pred, target)
    sim = MultiCoreSim(nc, N_CORES)
    for c in range(N_CORES):
        for name, arr in in_maps[c].items():
            sim.cores[c].tensor(name)[:] = arr
    sim.simulate()
    results = [{"acc_out": np.array(sim.cores[c].tensor("acc_out"))}
               for c in range(N_CORES)]
    return _reduce(results), sim.global_time


# revision 49
# speedup vs baseline: 1.4057x; 1.0066x over previous
"""ColorHistogramLoss Trainium2 kernel (v3).

Math: reference soft-histogram weight for pixel x and bin k is
    w = exp(-(x - c_k)^2 / (2 sigma^2)),  sigma = bin_width = 1/64, c_k = (k+0.5)/64
In bin units u = 64x.  With y = x - 0.5 (exact in fp32) and e_k = (k+0.5) - 32:
    t = 64y - e_k,   t^2/2 = 2048 y^2 - 64 e_k y + e_k^2/2
So per (pixel, bin):
    w = Exp( -(2048 y^2 - 64 e_k y) - e_k^2/2 )
The quadratic form rides the TensorEngine as a K=32 constant-stationary matmul
(rows = bf16 hi/lo splits of [y, y^2] for the two images of a pair; 128 PSUM
partitions = 2 images x 64 bins), then one ScalarEngine Exp pass per 2048-px
chunk (per-partition bias -e_k^2/2, in-place in PSUM, fused accum_out) yields
per-chunk bin sums.  Host folds partials in fp64, cumsums, and takes the loss.

Pipeline: the 24 images are processed in 3 thirds of 8 images (one 32-row
feature block each).  Per third: load wide slice -> DVE features (bf16 hi/lo)
-> store to DRAM in pair-major layout -> reload column-chunks -> matmul+Exp.
Thirds overlap: third T+1 preps while third T streams through PE/ACT.

Sharding: each of the 8 cores processes a 1/8 pixel-slice of all 24 images
(12 pred + 12 target); partial histogram sums are combined on host.
"""

import os

import numpy as np

N_CORES = 8
B, C, H, W = 4, 3, 256, 256
NIMG = 2 * B * C          # 24 images (12 pred + 12 target)
NPX = H * W               # 65536 pixels / image
PXC = NPX // N_CORES      # 8192 pixels / image / core
NPAIR = NIMG // 2         # 12 image pairs (2 imgs x 64 bins = 128 partitions)
NTHIRD = 3                # thirds of 8 images = one 32-partition block each
TCOL = 512                # wide-layout columns per third (8 imgs x 64)
WIDE_F = NTHIRD * TCOL    # 1536 wide-layout free dim
CHUNK = 512               # pixels per matmul (one PSUM bank)
ACHUNK = 2048             # pixels per ACT op (4 PSUM banks, double buffered)
NCH = PXC // ACHUNK       # 4 ACT chunks per pair per core
NCOL = NPAIR * NCH        # 48 accumulator columns
BINS = 64

_CACHE = {}


def _consts():
    k = np.arange(128) % 64
    e = (k + 0.5) - 32.0
    half = np.arange(128) // 64       # which image of the pair
    # per-pair stationary: pair j lives at partition block 32(j//4) (same
    # base partition as its rhs rows), rows 8(j%4)+4h+f: feature f of
    # pair-image h (f=0,1: y hi/lo -> -64 e_k; f=2,3: y^2 hi/lo -> 2048),
    # nonzero only on that half's bin columns; other slots' rows stay 0.
    stat = np.zeros((96, NPAIR * 128), np.float32)
    for j in range(NPAIR):
        rb = 32 * (j // 4) + 8 * (j % 4)
        for h in (0, 1):
            cols = np.where(half == h)[0] + 128 * j
            stat[rb + 4 * h + 0, cols] = -64.0 * e[half == h]
            stat[rb + 4 * h + 1, cols] = -64.0 * e[half == h]
            stat[rb + 4 * h + 2, cols] = 2048.0
            stat[rb + 4 * h + 3, cols] = 2048.0
    # col 0: per-bin exp bias -e^2/2; col 1: the -0.5 recentering constant
    biasd = np.stack([-(e * e) / 2.0, np.full(128, -0.5)],
                     axis=1).astype(np.float32)
    return stat, biasd


def _build():
    import concourse.bacc as bacc
    import concourse.tile as tile
    import concourse.mybir as mybir

    f32 = mybir.dt.float32
    bf16 = mybir.dt.bfloat16
    nc = bacc.Bacc("TRN2", target_bir_lowering=False, debug=False,
                   num_devices=N_CORES)

    # wide layout: partition p, col 64i+c  <->  image i, pixel 64p+c
    xin = nc.dram_tensor("xin", [128, WIDE_F], f32, kind="ExternalInput")
    stat = nc.dram_tensor("stat", [96, NPAIR * 128], bf16,
                          kind="ExternalInput")
    biasd = nc.dram_tensor("biasd", [128, 2], f32, kind="ExternalInput")
    prep = nc.dram_tensor("prep", [96, PXC], bf16)
    out = nc.dram_tensor("acc_out", [128, NCOL + 2], f32,
                         kind="ExternalOutput")

    with tile.TileContext(nc) as tc:
        with (
            tc.tile_pool(name="p_const", bufs=1) as cpool,
            tc.tile_pool(name="p_wide", bufs=2) as wpool,
            tc.tile_pool(name="p_feat", bufs=3) as fpool,
            tc.tile_pool(name="p_pair", bufs=1) as ppool,
            tc.tile_pool(name="p_acc", bufs=1) as apool,
            tc.tile_pool(name="p_psum", bufs=2, space="PSUM") as qpool,
            tc.tile_pool(name="p_exq", bufs=6) as epool,
        ):
            # first wide input slice goes first so the DVE chain starts ASAP
            xws = []
            xw = wpool.tile([128, TCOL], f32, tag="xw0")
            nc.sync.dma_start(out=xw[:], in_=xin[:, 0:TCOL])
            xws.append(xw)

            stat_t = cpool.tile([96, NPAIR * 128], bf16)
            nc.sync.dma_start(out=stat_t[:], in_=stat[:])
            bias_t = cpool.tile([128, 2], f32)
            nc.sync.dma_start(out=bias_t[:], in_=biasd[:])
            # pre-warm the exp table so ACT_TABLE_LOAD is off the critical path
            warm = cpool.tile([128, 1], f32)
            nc.scalar.activation(out=warm[:], in_=bias_t[:, 0:1],
                                 func=mybir.ActivationFunctionType.Exp)

            acc = apool.tile([128, NCOL + 2], f32)
            # pt row 4i+f = feature f (yh,yl,sh,sl) of image i
            pt = ppool.tile([96, PXC], bf16)
            # prep DRAM view for the pair-major store: [f][p, i, c]
            prep_v = prep[:].rearrange("(i four) (p c) -> four p i c",
                                       four=4, p=128)

            # prefetch the remaining wide input slices
            for T in range(1, NTHIRD):
                xw = wpool.tile([128, TCOL], f32, tag=f"xw{T % 2}")
                nc.sync.dma_start(out=xw[:],
                                  in_=xin[:, TCOL * T:TCOL * (T + 1)])
                xws.append(xw)

            for T in range(NTHIRD):
                xw = xws[T]
                y = wpool.tile([128, TCOL], f32, tag=f"y{T % 2}")
                nc.vector.tensor_scalar_add(out=y[:], in0=xw[:], scalar1=-0.5)
                y2 = wpool.tile([128, TCOL], f32, tag=f"y2{T % 2}")
                nc.vector.tensor_mul(out=y2[:], in0=y[:], in1=y[:])
                # feature block: f at cols [512f, 512f+512); hi/lo bf16 splits
                # (v = hi + lo; products are exact in fp32 PSUM accumulation)
                fg = fpool.tile([128, 4 * TCOL], bf16, tag="fg")
                nc.vector.tensor_copy(out=fg[:, 0:TCOL], in_=y[:])
                nc.vector.tensor_sub(out=fg[:, TCOL:2 * TCOL], in0=y[:],
                                     in1=fg[:, 0:TCOL])
                nc.vector.tensor_copy(out=fg[:, 2 * TCOL:3 * TCOL], in_=y2[:])
                nc.vector.tensor_sub(out=fg[:, 3 * TCOL:4 * TCOL], in0=y2[:],
                                     in1=fg[:, 2 * TCOL:3 * TCOL])
                if T > 0:
                    # dependency gate: fg[0, f*TCOL] = (pt_prev * 0) + itself
                    # is an exact identity on one element of each feature
                    # block, but makes this third's stores wait for the
                    # previous third's first pt load — keeps them off the
                    # DMA device while the critical-path load is pending
                    p0 = 32 * (T - 1)   # same start partition as the pt read
                    gv = fg[p0:p0 + 1, 0:3 * TCOL + 1:TCOL]
                    nc.vector.scalar_tensor_tensor(
                        out=gv, in0=pt[p0:p0 + 1, 0:4],
                        scalar=0.0, in1=gv,
                        op0=mybir.AluOpType.mult, op1=mybir.AluOpType.add)
                # store features to DRAM in pair-major row layout (row 4i+f)
                for f in range(4):
                    nc.sync.dma_start(
                        out=prep_v[f][:, 8 * T:8 * (T + 1), :],
                        in_=fg[:, TCOL * f:TCOL * (f + 1)].rearrange(
                            "p (i c) -> p i c", c=64),
                    )
                # reload as [32-row block, column chunks] for the matmuls;
                # the very first chunk is further split 512+1536 so the
                # first matmul/ACT can launch as early as possible
                r0 = 32 * T
                if T == 0:
                    for cs in (slice(0, CHUNK), slice(CHUNK, 2 * CHUNK),
                               slice(2 * CHUNK, ACHUNK)):
                        nc.sync.dma_start(out=pt[r0:r0 + 32, cs],
                                          in_=prep[r0:r0 + 32, cs])
                else:
                    nc.sync.dma_start(out=pt[r0:r0 + 32, 0:ACHUNK],
                                      in_=prep[r0:r0 + 32, 0:ACHUNK])
                for ch in range(1, NCH):
                    cs = slice(ACHUNK * ch, ACHUNK * (ch + 1))
                    nc.sync.dma_start(out=pt[r0:r0 + 32, cs],
                                      in_=prep[r0:r0 + 32, cs])
            for T in range(NTHIRD):
                r0 = 32 * T
                for ch in range(NCH):
                    for js in range(4):
                        j = 4 * T + js
                        ps = qpool.tile([128, ACHUNK], f32, tag="ps")
                        head = T == 0 and ch == 0 and js == 0
                        for h in range(ACHUNK // CHUNK):
                            px0 = ACHUNK * ch + CHUNK * h
                            nc.tensor.matmul(
                                out=ps[:, CHUNK * h:CHUNK * (h + 1)],
                                lhsT=stat_t[r0:r0 + 32,
                                            128 * j:128 * (j + 1)],
                                rhs=pt[r0:r0 + 32, px0:px0 + CHUNK],
                                start=True, stop=True,
                            )
                            if head and h < 2:
                                # head ops: Exp the first 512-px pieces right
                                # away (extra accum cols, folded on host) so
                                # the ACT stream starts as early as possible
                                nc.scalar.activation(
                                    out=ps[:, CHUNK * h:CHUNK * (h + 1)],
                                    in_=ps[:, CHUNK * h:CHUNK * (h + 1)],
                                    func=mybir.ActivationFunctionType.Exp,
                                    bias=bias_t[:, 0:1], scale=-1.0,
                                    accum_out=acc[:, NCOL + h:NCOL + h + 1],
                                )
                        col = NCH * j + ch
                        lo = 2 * CHUNK if head else 0
                        if js == 3 or (T == 2 and ch == 3 and js == 2):
                            # 1-in-4: fused accumulate on the ScalarEngine
                            # (last in each group — and both tail ops of the
                            # whole stream — so no DVE reduce trails the end)
                            nc.scalar.activation(
                                out=ps[:, lo:ACHUNK], in_=ps[:, lo:ACHUNK],
                                func=mybir.ActivationFunctionType.Exp,
                                bias=bias_t[:, 0:1], scale=-1.0,
                                accum_out=acc[:, col:col + 1],
                            )
                        else:
                            # 3-in-4: Exp to SBUF bf16, pixel-sum on the DVE;
                            # skipping the ACT accumulator read shortens the
                            # ScalarEngine op from 2037 to 1892 ns
                            exq = epool.tile([128, ACHUNK], bf16, tag="exq")
                            nc.scalar.activation(
                                out=exq[:, 0:ACHUNK - lo], in_=ps[:, lo:ACHUNK],
                                func=mybir.ActivationFunctionType.Exp,
                                bias=bias_t[:, 0:1], scale=-1.0,
                            )
                            nc.vector.reduce_sum(
                                out=acc[:, col:col + 1],
                                in_=exq[:, 0:ACHUNK - lo],
                                axis=mybir.AxisListType.X,
                            )
                # drain the first two blocks' accumulator columns early so
                # only a tiny output DMA remains after the final Exp
                if T == 1:
                    nc.sync.dma_start(out=out[:, 0:32], in_=acc[:, 0:32])
            nc.sync.dma_start(out=out[:, 32:NCOL + 2],
                              in_=acc[:, 32:NCOL + 2])
    if not nc.is_finalized():
        nc.finalize()
    return nc


def _in_maps(pred, target):
    X = np.concatenate(
        [np.asarray(pred, np.float32).reshape(B * C, NPX),
         np.asarray(target, np.float32).reshape(B * C, NPX)], axis=0)
    statM, biasv = _consts()
    from ml_dtypes import bfloat16 as np_bf16
    statM = statM.astype(np_bf16)
    maps = []
    for cs in range(N_CORES):
        Xs = X[:, cs * PXC:(cs + 1) * PXC]              # [24, 8192]
        xin = np.ascontiguousarray(
            Xs.reshape(NIMG, 128, 64)                   # [i, p, c]
              .transpose(1, 0, 2)                       # [p, i, c]
              .reshape(128, WIDE_F))
        maps.append({"xin": xin, "stat": statM, "biasd": biasv})
    return maps


def _reduce(results):
    A = np.stack([r["acc_out"] for r in results]).astype(np.float64)
    A = A.sum(axis=0)
    A[:, 0] += A[:, NCOL] + A[:, NCOL + 1]   # fold head partials (j=0,ch=0)
    M = A[:, :NCOL].reshape(128, NPAIR, NCH).sum(axis=2)     # [128, 12]
    Hh = np.empty((NIMG, BINS), np.float64)
    for j in range(NPAIR):
        Hh[2 * j] = M[:64, j]
        Hh[2 * j + 1] = M[64:, j]
    cum = np.cumsum(Hh, axis=1)
    den = cum[:, -1:] + 1e-8
    cdf = cum / den
    loss = np.mean(np.abs(cdf[:B * C] - cdf[B * C:]))
    return np.array(loss, dtype=np.float32)


def kernel(pred: np.ndarray, target: np.ndarray) -> np.ndarray:
    if "nc" not in _CACHE:
        _CACHE["nc"] = _build()
    nc = _CACHE["nc"]
    in_maps = _in_maps(pred, target)

    from concourse.bass_utils import run_bass_kernel_spmd
    trace = bool(int(os.environ.get("KERNEL_TRACE", "0")))
    res = run_bass_kernel_spmd(nc, in_maps, core_ids=list(range(N_CORES)),
                               trace=trace)
    if res.exec_time_ns:
        _CACHE["exec_time_ns"] = res.exec_time_ns
    return _reduce(res.results)


def kernel_sim(pred: np.ndarray, target: np.ndarray):
    """Run through the CoreSim timing simulator; returns (loss, sim_ns)."""
    from concourse.bass_interp import MultiCoreSim
    nc = _build()
    in_maps = _in_maps(pred, target)
    sim = MultiCoreSim(nc, N_CORES)
    for c in range(N_CORES):
        for name, arr in in_maps[c].items():
            sim.cores[c].tensor(name)[:] = arr
    sim.simulate()
    results = [{"acc_out": np.array(sim.cores[c].tensor("acc_out"))}
               for c in range(N_CORES)]
    return _reduce(results), sim.global_time


# revision 56
# speedup vs baseline: 1.4095x; 1.0027x over previous
"""ColorHistogramLoss Trainium2 kernel (v3).

Math: reference soft-histogram weight for pixel x and bin k is
    w = exp(-(x - c_k)^2 / (2 sigma^2)),  sigma = bin_width = 1/64, c_k = (k+0.5)/64
In bin units u = 64x.  With y = x - 0.5 (exact in fp32) and e_k = (k+0.5) - 32:
    t = 64y - e_k,   t^2/2 = 2048 y^2 - 64 e_k y + e_k^2/2
So per (pixel, bin):
    w = Exp( -(2048 y^2 - 64 e_k y) - e_k^2/2 )
The quadratic form rides the TensorEngine as a K=32 constant-stationary matmul
(rows = bf16 hi/lo splits of [y, y^2] for the two images of a pair; 128 PSUM
partitions = 2 images x 64 bins), then one ScalarEngine Exp pass per 2048-px
chunk (per-partition bias -e_k^2/2, in-place in PSUM, fused accum_out) yields
per-chunk bin sums.  Host folds partials in fp64, cumsums, and takes the loss.

Pipeline: the 24 images are processed in 3 thirds of 8 images (one 32-row
feature block each).  Per third: load wide slice -> DVE features (bf16 hi/lo)
-> store to DRAM in pair-major layout -> reload column-chunks -> matmul+Exp.
Thirds overlap: third T+1 preps while third T streams through PE/ACT.

Sharding: each of the 8 cores processes a 1/8 pixel-slice of all 24 images
(12 pred + 12 target); partial histogram sums are combined on host.
"""

import os

import numpy as np

N_CORES = 8
B, C, H, W = 4, 3, 256, 256
NIMG = 2 * B * C          # 24 images (12 pred + 12 target)
NPX = H * W               # 65536 pixels / image
PXC = NPX // N_CORES      # 8192 pixels / image / core
NPAIR = NIMG // 2         # 12 image pairs (2 imgs x 64 bins = 128 partitions)
NTHIRD = 3                # thirds of 8 images = one 32-partition block each
TCOL = 512                # wide-layout columns per third (8 imgs x 64)
WIDE_F = NTHIRD * TCOL    # 1536 wide-layout free dim
CHUNK = 512               # pixels per matmul (one PSUM bank)
ACHUNK = 2048             # pixels per ACT op (4 PSUM banks, double buffered)
NCH = PXC // ACHUNK       # 4 ACT chunks per pair per core
NCOL = NPAIR * NCH        # 48 accumulator columns
BINS = 64

_CACHE = {}


def _consts():
    k = np.arange(128) % 64
    e = (k + 0.5) - 32.0
    half = np.arange(128) // 64       # which image of the pair
    # per-pair stationary: pair j lives at partition block 32(j//4) (same
    # base partition as its rhs rows), rows 8(j%4)+4h+f: feature f of
    # pair-image h (f=0,1: y hi/lo -> -64 e_k; f=2,3: y^2 hi/lo -> 2048),
    # nonzero only on that half's bin columns; other slots' rows stay 0.
    stat = np.zeros((96, NPAIR * 128), np.float32)
    for j in range(NPAIR):
        rb = 32 * (j // 4) + 8 * (j % 4)
        for h in (0, 1):
            cols = np.where(half == h)[0] + 128 * j
            stat[rb + 4 * h + 0, cols] = -64.0 * e[half == h]
            stat[rb + 4 * h + 1, cols] = -64.0 * e[half == h]
            stat[rb + 4 * h + 2, cols] = 2048.0
            stat[rb + 4 * h + 3, cols] = 2048.0
    # col 0: per-bin exp bias -e^2/2; col 1: the -0.5 recentering constant
    biasd = np.stack([-(e * e) / 2.0, np.full(128, -0.5)],
                     axis=1).astype(np.float32)
    return stat, biasd


def _build():
    import concourse.bacc as bacc
    import concourse.tile as tile
    import concourse.mybir as mybir

    f32 = mybir.dt.float32
    bf16 = mybir.dt.bfloat16
    nc = bacc.Bacc("TRN2", target_bir_lowering=False, debug=False,
                   num_devices=N_CORES)

    # wide layout: partition p, col 64i+c  <->  image i, pixel 64p+c
    xin = nc.dram_tensor("xin", [128, WIDE_F], f32, kind="ExternalInput")
    stat = nc.dram_tensor("stat", [96, NPAIR * 128], bf16,
                          kind="ExternalInput")
    biasd = nc.dram_tensor("biasd", [128, 2], f32, kind="ExternalInput")
    prep = nc.dram_tensor("prep", [96, PXC], bf16)
    out = nc.dram_tensor("acc_out", [128, NCOL + 2], f32,
                         kind="ExternalOutput")

    with tile.TileContext(nc) as tc:
        with (
            tc.tile_pool(name="p_const", bufs=1) as cpool,
            tc.tile_pool(name="p_wide", bufs=2) as wpool,
            tc.tile_pool(name="p_feat", bufs=3) as fpool,
            tc.tile_pool(name="p_pair", bufs=1) as ppool,
            tc.tile_pool(name="p_acc", bufs=1) as apool,
            tc.tile_pool(name="p_psum", bufs=2, space="PSUM") as qpool,
            tc.tile_pool(name="p_exq", bufs=6) as epool,
        ):
            # first wide input slice goes first so the DVE chain starts ASAP
            xws = []
            xw = wpool.tile([128, TCOL], f32, tag="xw0")
            nc.sync.dma_start(out=xw[:], in_=xin[:, 0:TCOL])
            xws.append(xw)

            stat_t = cpool.tile([96, NPAIR * 128], bf16)
            nc.sync.dma_start(out=stat_t[:], in_=stat[:])
            bias_t = cpool.tile([128, 2], f32)
            nc.sync.dma_start(out=bias_t[:], in_=biasd[:])
            # pre-warm the exp table so ACT_TABLE_LOAD is off the critical path
            warm = cpool.tile([128, 1], f32)
            nc.scalar.activation(out=warm[:], in_=bias_t[:, 0:1],
                                 func=mybir.ActivationFunctionType.Exp)

            acc = apool.tile([128, NCOL + 2], f32)
            # pt row 4i+f = feature f (yh,yl,sh,sl) of image i
            pt = ppool.tile([96, PXC], bf16)
            # prep DRAM view for the pair-major store: [f][p, i, c]
            prep_v = prep[:].rearrange("(i four) (p c) -> four p i c",
                                       four=4, p=128)

            # prefetch the remaining wide input slices
            for T in range(1, NTHIRD):
                xw = wpool.tile([128, TCOL], f32, tag=f"xw{T % 2}")
                nc.sync.dma_start(out=xw[:],
                                  in_=xin[:, TCOL * T:TCOL * (T + 1)])
                xws.append(xw)

            for T in range(NTHIRD):
                xw = xws[T]
                y = wpool.tile([128, TCOL], f32, tag=f"y{T % 2}")
                y2 = wpool.tile([128, TCOL], f32, tag=f"y2{T % 2}")
                # feature block: f at cols [512f, 512f+512); hi/lo bf16 splits
                # (v = hi + lo; products are exact in fp32 PSUM accumulation)
                fg = fpool.tile([128, 4 * TCOL], bf16, tag="fg")
                for hs in (slice(0, TCOL),):
                    nc.vector.tensor_scalar_add(out=y[:, hs], in0=xw[:, hs],
                                                scalar1=-0.5)
                    nc.vector.tensor_mul(out=y2[:, hs], in0=y[:, hs],
                                         in1=y[:, hs])
                    for f, src in ((0, y), (2, y2)):
                        lo = f * TCOL + hs.start
                        hi = f * TCOL + hs.stop
                        nc.vector.tensor_copy(out=fg[:, lo:hi],
                                              in_=src[:, hs])
                        nc.vector.tensor_sub(
                            out=fg[:, TCOL + lo:TCOL + hi],
                            in0=src[:, hs], in1=fg[:, lo:hi])
                if T > 0:
                    # dependency gate: fg[0, f*TCOL] = (pt_prev * 0) + itself
                    # is an exact identity on one element of each feature
                    # block, but makes this third's stores wait for the
                    # previous third's first pt load — keeps them off the
                    # DMA device while the critical-path load is pending
                    p0 = 32 * (T - 1)   # same start partition as the pt read
                    gv = fg[p0:p0 + 1, 0:3 * TCOL + 1:TCOL]
                    nc.vector.scalar_tensor_tensor(
                        out=gv, in0=pt[p0:p0 + 1, 0:4],
                        scalar=0.0, in1=gv,
                        op0=mybir.AluOpType.mult, op1=mybir.AluOpType.add)
                # store features to DRAM in pair-major row layout (row 4i+f)
                for f in range(4):
                    nc.sync.dma_start(
                        out=prep_v[f][:, 8 * T:8 * (T + 1), :],
                        in_=fg[:, TCOL * f:TCOL * (f + 1)].rearrange(
                            "p (i c) -> p i c", c=64),
                    )
                # reload as [32-row block, column chunks] for the matmuls;
                # the very first chunk is further split 512+1536 so the
                # first matmul/ACT can launch as early as possible
                r0 = 32 * T
                if T == 0:
                    for cs in (slice(0, CHUNK), slice(CHUNK, 2 * CHUNK),
                               slice(2 * CHUNK, ACHUNK)):
                        nc.sync.dma_start(out=pt[r0:r0 + 32, cs],
                                          in_=prep[r0:r0 + 32, cs])
                else:
                    nc.sync.dma_start(out=pt[r0:r0 + 32, 0:ACHUNK],
                                      in_=prep[r0:r0 + 32, 0:ACHUNK])
                for ch in range(1, NCH):
                    cs = slice(ACHUNK * ch, ACHUNK * (ch + 1))
                    nc.sync.dma_start(out=pt[r0:r0 + 32, cs],
                                      in_=prep[r0:r0 + 32, cs])
            for T in range(NTHIRD):
                r0 = 32 * T
                for ch in range(NCH):
                    for js in range(4):
                        j = 4 * T + js
                        ps = qpool.tile([128, ACHUNK], f32, tag="ps")
                        head = T == 0 and ch == 0 and js == 0
                        for h in range(ACHUNK // CHUNK):
                            px0 = ACHUNK * ch + CHUNK * h
                            nc.tensor.matmul(
                                out=ps[:, CHUNK * h:CHUNK * (h + 1)],
                                lhsT=stat_t[r0:r0 + 32,
                                            128 * j:128 * (j + 1)],
                                rhs=pt[r0:r0 + 32, px0:px0 + CHUNK],
                                start=True, stop=True,
                            )
                            if head and h < 2:
                                # head ops: Exp the first 512-px pieces right
                                # away (extra accum cols, folded on host) so
                                # the ACT stream starts as early as possible
                                nc.scalar.activation(
                                    out=ps[:, CHUNK * h:CHUNK * (h + 1)],
                                    in_=ps[:, CHUNK * h:CHUNK * (h + 1)],
                                    func=mybir.ActivationFunctionType.Exp,
                                    bias=bias_t[:, 0:1], scale=-1.0,
                                    accum_out=acc[:, NCOL + h:NCOL + h + 1],
                                )
                        col = NCH * j + ch
                        lo = 2 * CHUNK if head else 0
                        if (js == 3 and not (T == 0 and ch <= 1)) or (
                                T == 2 and ch == 3 and js == 2):
                            # 1-in-4: fused accumulate on the ScalarEngine
                            # (last in each group — and both tail ops of the
                            # whole stream — so no DVE reduce trails the end)
                            nc.scalar.activation(
                                out=ps[:, lo:ACHUNK], in_=ps[:, lo:ACHUNK],
                                func=mybir.ActivationFunctionType.Exp,
                                bias=bias_t[:, 0:1], scale=-1.0,
                                accum_out=acc[:, col:col + 1],
                            )
                        else:
                            # 3-in-4: Exp to SBUF bf16, pixel-sum on the DVE;
                            # skipping the ACT accumulator read shortens the
                            # ScalarEngine op from 2037 to 1892 ns
                            exq = epool.tile([128, ACHUNK], bf16, tag="exq")
                            nc.scalar.activation(
                                out=exq[:, 0:ACHUNK - lo], in_=ps[:, lo:ACHUNK],
                                func=mybir.ActivationFunctionType.Exp,
                                bias=bias_t[:, 0:1], scale=-1.0,
                            )
                            nc.vector.reduce_sum(
                                out=acc[:, col:col + 1],
                                in_=exq[:, 0:ACHUNK - lo],
                                axis=mybir.AxisListType.X,
                            )
                # drain the first two blocks' accumulator columns early so
                # only a tiny output DMA remains after the final Exp
                if T == 1:
                    nc.sync.dma_start(out=out[:, 0:32], in_=acc[:, 0:32])
            # issue the tail DMA from the ACT engine's HWDGE ring: it follows
            # the final Exp on the same engine, skipping a cross-engine hop
            nc.scalar.dma_start(out=out[:, 32:NCOL + 2],
                                in_=acc[:, 32:NCOL + 2])
    if not nc.is_finalized():
        nc.finalize()
    return nc


def _in_maps(pred, target):
    X = np.concatenate(
        [np.asarray(pred, np.float32).reshape(B * C, NPX),
         np.asarray(target, np.float32).reshape(B * C, NPX)], axis=0)
    statM, biasv = _consts()
    from ml_dtypes import bfloat16 as np_bf16
    statM = statM.astype(np_bf16)
    maps = []
    for cs in range(N_CORES):
        Xs = X[:, cs * PXC:(cs + 1) * PXC]              # [24, 8192]
        xin = np.ascontiguousarray(
            Xs.reshape(NIMG, 128, 64)                   # [i, p, c]
              .transpose(1, 0, 2)                       # [p, i, c]
              .reshape(128, WIDE_F))
        maps.append({"xin": xin, "stat": statM, "biasd": biasv})
    return maps


def _reduce(results):
    A = np.stack([r["acc_out"] for r in results]).astype(np.float64)
    A = A.sum(axis=0)
    A[:, 0] += A[:, NCOL] + A[:, NCOL + 1]   # fold head partials (j=0,ch=0)
    M = A[:, :NCOL].reshape(128, NPAIR, NCH).sum(axis=2)     # [128, 12]
    Hh = np.empty((NIMG, BINS), np.float64)
    for j in range(NPAIR):
        Hh[2 * j] = M[:64, j]
        Hh[2 * j + 1] = M[64:, j]
    cum = np.cumsum(Hh, axis=1)
    den = cum[:, -1:] + 1e-8
    cdf = cum / den
    loss = np.mean(np.abs(cdf[:B * C] - cdf[B * C:]))
    return np.array(loss, dtype=np.float32)


def kernel(pred: np.ndarray, target: np.ndarray) -> np.ndarray:
    if "nc" not in _CACHE:
        _CACHE["nc"] = _build()
    nc = _CACHE["nc"]
    in_maps = _in_maps(pred, target)

    from concourse.bass_utils import run_bass_kernel_spmd
    trace = bool(int(os.environ.get("KERNEL_TRACE", "0")))
    res = run_bass_kernel_spmd(nc, in_maps, core_ids=list(range(N_CORES)),
                               trace=trace)
    if res.exec_time_ns:
        _CACHE["exec_time_ns"] = res.exec_time_ns
    return _reduce(res.results)


def kernel_sim(pred: np.ndarray, target: np.ndarray):
    """Run through the CoreSim timing simulator; returns (loss, sim_ns)."""
    from concourse.bass_interp import MultiCoreSim
    nc = _build()
    in_maps = _in_maps(pred, target)
    sim = MultiCoreSim(nc, N_CORES)
    for c in range(N_CORES):
        for name, arr in in_maps[c].items():
            sim.cores[c].tensor(name)[:] = arr
    sim.simulate()
    results = [{"acc_out": np.array(sim.cores[c].tensor("acc_out"))}
               for c in range(N_CORES)]
    return _reduce(results), sim.global_time
